# revision 3
# baseline (speedup 1.0000x reference)
"""Grouped per-expert SwiGLU FFN (MoE experts) on 8 TRN2 NeuronCores. v3.

Fused single-pipeline variant. Phase 1 (up/gate -> midT) unchanged from v2.
The down projection is restructured dsl-column-wise: out[:, dsl*512:...]
accumulates over all 64 h-tiles into 2 PSUM banks (t0,t1) per dsl, so the
down-proj needs only 4 PSUM banks (double-buffered across dsl) and can run
CONCURRENTLY with phase 1's 3 banks (7 of 8 total).

dsl=0's matmuls are interleaved into the phase-1 instruction stream with a
2-slice lag (midT[ht] is produced by DVE ~1-2us after the PE finishes
slice ht; the lag keeps the strict-FIFO PE queue from stalling on it).
dsl 1..3 run after phase 1 ends, streaming the remaining w2t columns.
w2t is loaded as column chunks [128, 16 ht, 512 d] (2 MiB, 1 KiB rows).

Down-proj drains (PSUM->SBUF->DRAM) of dsl overlap accumulation of dsl+1.
"""

import sys

if "/opt/trn_rl_repo" not in sys.path:
    sys.path.insert(0, "/opt/trn_rl_repo")

import numpy as np
import ml_dtypes

import concourse.mybir as mybir
import concourse.tile as tile
from concourse import bacc
from concourse.bass_utils import run_bass_kernel_spmd

E, T, D, H = 8, 256, 2048, 8192
P = 128
KD = D // P          # 16
HT = H // P          # 64
TT = T // P          # 2
H_SLICE = 512
HS = H // H_SLICE    # 16
D_SLICE = 512
DS = D // D_SLICE    # 4
HCH = 16             # w2t h-tiles per column chunk (2 MiB)
NCH = HT // HCH      # 4 chunks per dsl

BF16 = mybir.dt.bfloat16
F32 = mybir.dt.float32

_CACHED = {}

LAG = 2  # slices of delay before down-proj consumes midT


def _build(reps: int = 1):
    nc = bacc.Bacc("TRN2", target_bir_lowering=False, debug=False)
    xt_d = nc.dram_tensor("xt", [D, T], BF16, kind="ExternalInput").ap()
    w1_d = nc.dram_tensor("w1", [D, H], BF16, kind="ExternalInput").ap()
    w3_d = nc.dram_tensor("w3", [D, H], BF16, kind="ExternalInput").ap()
    w2t_d = nc.dram_tensor("w2t", [H, D], BF16, kind="ExternalInput").ap()
    out_d = nc.dram_tensor("out", [T, D], F32, kind="ExternalOutput").ap()

    xt_v = xt_d.rearrange("(o p) t -> p o t", p=P)
    w1_v = w1_d.rearrange("(o p) h -> p o h", p=P)
    w3_v = w3_d.rearrange("(o p) h -> p o h", p=P)
    w2t_v = w2t_d.rearrange("(o p) d -> p o d", p=P)    # [128, 64, 2048]
    out_v = out_d.rearrange("(o p) d -> p o d", p=P)    # [128, 2, 2048]

    with tile.TileContext(nc) as tc:
        with tc.tile_pool(name="persist", bufs=1) as cpool:
          for _rep in range(reps):
            xt_sb = cpool.tile([P, KD, T], BF16, tag="xt", name="xt_sb")
            midT = cpool.tile([P, HT, T], BF16, tag="midT", name="midT")

            nc.scalar.dma_start(xt_sb, xt_v)

            with (
                tc.tile_pool(name="wpool", bufs=3) as wpool,
                tc.tile_pool(name="w2pool", bufs=3) as w2pool,
                tc.tile_pool(name="act", bufs=3) as apool,
                tc.tile_pool(name="opool", bufs=4) as opool,
                tc.tile_pool(name="ps1", bufs=2, space="PSUM") as ps1,
                tc.tile_pool(name="ps2", bufs=2, space="PSUM") as ps2,
            ):
                # down-proj state: per dsl, psum tiles keyed (t, dsl%2)
                o_ps_cur = {}

                def o_ps_for(dsl):
                    return [
                        ps2.tile([P, D_SLICE], F32, tag=f"o{t}",
                                 name=f"o_ps_{t}_{dsl}")
                        for t in range(TT)
                    ]

                w2_tiles = {}  # (dsl, chunk) -> tile

                def w2_load(dsl, c):
                    w2_sb = w2pool.tile([P, HCH, D_SLICE], BF16, tag="w2",
                                        name="w2_sb")
                    eng = nc.sync if (dsl + c) % 2 == 0 else nc.scalar
                    dsl_sl = slice(dsl * D_SLICE, (dsl + 1) * D_SLICE)
                    eng.dma_start(
                        w2_sb, w2t_v[:, c * HCH:(c + 1) * HCH, dsl_sl])
                    w2_tiles[(dsl, c)] = w2_sb

                def down_mm(dsl, ht, o_ps):
                    w2_sb = w2_tiles[(dsl, ht // HCH)]
                    for t in range(TT):
                        tsl = slice(t * P, (t + 1) * P)
                        nc.tensor.matmul(
                            o_ps[t], midT[:, ht, tsl],
                            w2_sb[:, ht % HCH, :],
                            start=(ht == 0), stop=(ht == HT - 1),
                        )

                def drain(dsl, o_ps):
                    dslice = slice(dsl * D_SLICE, (dsl + 1) * D_SLICE)
                    for t in range(TT):
                        o_sb = opool.tile([P, D_SLICE], F32, tag="osb",
                                          name="o_sb")
                        nc.any.tensor_copy(out=o_sb, in_=o_ps[t])
                        (nc.sync if t % 2 == 0 else nc.scalar).dma_start(
                            out_v[:, t, dslice], o_sb)

                # ---- fused phase 1 + dsl0 down-proj ----
                o_ps_cur[0] = o_ps_for(0)
                w2_load(0, 0)
                for j in range(HS):
                    w1_sb = wpool.tile([P, KD, H_SLICE], BF16, tag="w1",
                                       name="w1_sb")
                    w3_sb = wpool.tile([P, KD, H_SLICE], BF16, tag="w3",
                                       name="w3_sb")
                    hsl = slice(j * H_SLICE, (j + 1) * H_SLICE)
                    nc.sync.dma_start(w1_sb, w1_v[:, :, hsl])
                    nc.scalar.dma_start(w3_sb, w3_v[:, :, hsl])
                    for s in range(H_SLICE // P):
                        ht = j * (H_SLICE // P) + s
                        ssl = slice(s * P, (s + 1) * P)
                        h1_ps = ps1.tile([P, T], F32, tag="h1", name="h1_ps")
                        h3_ps = ps1.tile([P, T], F32, tag="h3", name="h3_ps")
                        for kd in range(KD):
                            nc.tensor.matmul(
                                h1_ps, w1_sb[:, kd, ssl], xt_sb[:, kd, :],
                                start=(kd == 0), stop=(kd == KD - 1))
                        for kd in range(KD):
                            nc.tensor.matmul(
                                h3_ps, w3_sb[:, kd, ssl], xt_sb[:, kd, :],
                                start=(kd == 0), stop=(kd == KD - 1))
                        silu_sb = apool.tile([P, T], F32, tag="silu",
                                             name="silu_sb")
                        nc.scalar.activation(
                            silu_sb, h1_ps, mybir.ActivationFunctionType.Silu)
                        nc.vector.tensor_mul(out=midT[:, ht, :], in0=silu_sb,
                                             in1=h3_ps)
                        # trailing dsl0 down-proj, LAG slices behind
                        dht = ht - LAG
                        if dht >= 0:
                            if dht % HCH == 0 and dht // HCH + 1 < NCH:
                                w2_load(0, dht // HCH + 1)
                            down_mm(0, dht, o_ps_cur[0])
                # dsl0 leftovers
                for dht in range(HT - LAG, HT):
                    down_mm(0, dht, o_ps_cur[0])

                # ---- dsl 1..3 down-proj ----
                w2_load(1, 0)
                for dsl in range(1, DS):
                    o_ps_cur[dsl] = o_ps_for(dsl)
                    for c in range(NCH):
                        if c + 1 < NCH:
                            w2_load(dsl, c + 1)
                        elif dsl + 1 < DS:
                            w2_load(dsl + 1, 0)
                        for hh in range(HCH):
                            down_mm(dsl, c * HCH + hh, o_ps_cur[dsl])
                    drain(dsl - 1, o_ps_cur.pop(dsl - 1))
                drain(DS - 1, o_ps_cur.pop(DS - 1))

    nc.compile()
    return nc


def _get_nc():
    if "nc" not in _CACHED:
        _CACHED["nc"] = _build()
    return _CACHED["nc"]


def kernel(x, w1, w2, w3, **_unused):
    """x: [E,T,D] f32; w1,w2,w3: [E,D,H] f32 -> [E,T,D] f32."""
    bf = ml_dtypes.bfloat16
    in_maps = []
    for e in range(E):
        in_maps.append(
            {
                "xt": np.ascontiguousarray(np.asarray(x[e]).T).astype(bf),
                "w1": np.asarray(w1[e]).astype(bf),
                "w3": np.asarray(w3[e]).astype(bf),
                "w2t": np.ascontiguousarray(np.asarray(w2[e]).T).astype(bf),
            }
        )
    nc = _get_nc()
    res = run_bass_kernel_spmd(nc, in_maps, core_ids=list(range(E)))
    out = np.stack([res.results[e]["out"] for e in range(E)], axis=0)
    return out.astype(np.float32, copy=False)
